# Initial kernel scaffold
#
"""Trainium2 Bass kernel for nn_Decoder_58377195487266.

Single-layer decoder: shared-head causal attention (d_k=32) + FFN(256->1024->256)
with two LayerNorms. B=16, T=2048, EMB=256.

Sharding: pure data-parallel over batch. 8 cores x 2 batches each, weights
replicated, no collectives.

Host-side algebraic folds (exact):
  - cat = tile(attn, 8)  =>  cat @ Wp == attn @ (sum of Wp's 8 row blocks).
  - 1/sqrt(d_k) score scale folded into Wq.
  - bp folded into the residual input (x + bp).
  - softmax denominator: ones-column appended to v + unit column appended to
    the folded Wp, so the mh matmul also emits sum_s exp(score) per token;
    normalization multiplies by its reciprocal during z1 assembly.
"""

import math
import os

import numpy as np

import concourse.bass as bass  # noqa: F401
import concourse.mybir as mybir
from concourse import bacc
from concourse.bass_utils import run_bass_kernel_spmd
from concourse.masks import make_identity
from concourse.tile import TileContext

F32 = mybir.dt.float32
F32R = mybir.dt.float32r
I32 = mybir.dt.int32
AF = mybir.ActivationFunctionType
OP = mybir.AluOpType

B, T, EMB = 16, 2048, 256
DK = 32
HID = 4 * EMB
N_CORES = 8
B_LOC = B // N_CORES  # 2
NT = T // 128         # 16 t-blocks
NCH = T // 512        # 4 chunks
LN_EPS = 1e-5

RSQRT_MAGIC = 0x5F3759DF


def _nr_rsqrt(nc, pool, out, varp):
    """out = 1/sqrt(varp) via bit-trick + 3 Newton iterations, all on DVE."""
    sh = list(varp.shape)
    yi = pool.tile(sh, I32, tag="nr_i", bufs=2)
    magic = pool.tile(sh, I32, tag="nr_m", bufs=2)
    nc.vector.memset(magic[:], RSQRT_MAGIC)
    nc.vector.tensor_scalar(yi[:], varp.bitcast(I32), 1, None, OP.logical_shift_right)
    nc.vector.tensor_tensor(yi[:], magic[:], yi[:], OP.subtract)
    y = yi.bitcast(F32)
    e = pool.tile(sh, F32, tag="nr_e", bufs=2)
    h = pool.tile(sh, F32, tag="nr_h", bufs=2)
    for _ in range(3):
        nc.vector.tensor_tensor(e[:], y[:], y[:], OP.mult)
        nc.vector.tensor_tensor(e[:], e[:], varp[:], OP.mult)
        nc.vector.tensor_scalar(h[:], e[:], -0.5, 1.5, OP.mult, OP.add)
        nc.vector.tensor_tensor(y[:], y[:], h[:], OP.mult)
    nc.vector.tensor_copy(out[:], y[:])


def _layernorm(nc, st_pool, sm_pool, z_sb, y_sb):
    """Natural-layout LN: stats from accumulated sums, NR rsqrt, gpsimd apply.

    z_sb: [128, NT, EMB] fp32. Writes normalized (no affine) y_sb.
    """
    stats6 = st_pool.tile([128, NT, 6], F32, tag="st6", bufs=2)
    agg = st_pool.tile([128, NT, 2], F32, tag="agg", bufs=2)
    for tb in range(NT):
        nc.vector.bn_stats(stats6[:, tb], z_sb[:, tb])
        nc.vector.bn_aggr(agg[:, tb], stats6[:, tb])
    mean = agg[:, :, 0]
    varp = st_pool.tile([128, NT], F32, tag="varp", bufs=2)
    rstd = st_pool.tile([128, NT], F32, tag="rstd", bufs=2)
    mrstd = st_pool.tile([128, NT], F32, tag="mrstd", bufs=2)
    nc.vector.tensor_scalar(varp[:], agg[:, :, 1], 1.0, LN_EPS, OP.mult, OP.add)
    _nr_rsqrt(nc, st_pool, rstd, varp)
    nc.vector.tensor_tensor(mrstd[:], mean, rstd[:], OP.mult)
    for tb in range(NT):
        eng = nc.vector if os.environ.get("KDBG_NO_GPSIMD") else nc.gpsimd
        eng.tensor_scalar(
            y_sb[:, tb], z_sb[:, tb],
            rstd[:, tb:tb + 1], mrstd[:, tb:tb + 1],
            OP.mult, OP.subtract,
        )


def build_decoder(apply_g1be1: bool, apply_g2be2: bool, apply_b2: bool):
    """Build the per-core Bass program (B_LOC batches, full T each)."""
    PH = int(os.environ.get("KDBG_PHASE", "99"))
    nc = bacc.Bacc(None, target_bir_lowering=False)

    xp_d = nc.dram_tensor("xp", [B_LOC, T, EMB], F32, kind="ExternalInput")
    xt_d = nc.dram_tensor("xt", [B_LOC, EMB, T], F32, kind="ExternalInput")
    wq4_d = nc.dram_tensor("wq4", [EMB, 128], F32, kind="ExternalInput")
    wk4_d = nc.dram_tensor("wk4", [EMB, 128], F32, kind="ExternalInput")
    wv_d = nc.dram_tensor("wv", [EMB, DK], F32, kind="ExternalInput")
    wpf_d = nc.dram_tensor("wpf", [DK + 2, EMB + 2], F32, kind="ExternalInput")
    w1_d = nc.dram_tensor("w1", [EMB, HID], F32, kind="ExternalInput")
    b1_d = nc.dram_tensor("b1", [128, 8], F32, kind="ExternalInput")
    w2_d = nc.dram_tensor("w2", [HID, EMB], F32, kind="ExternalInput")
    aff_d = nc.dram_tensor("aff", [1, 5, EMB], F32, kind="ExternalInput")
    # aff rows: b2, g1, be1, g2, be2
    out_d = nc.dram_tensor("out", [B_LOC, T, EMB], F32, kind="ExternalOutput")

    need_bcast = apply_g1be1 or apply_g2be2

    with TileContext(nc) as tc:
        with (
            tc.tile_pool(name="wpool", bufs=1) as wp,
            tc.tile_pool(name="xpool", bufs=2) as xq,
            tc.tile_pool(name="qkpool", bufs=1) as qk_pool,
            tc.tile_pool(name="atpool", bufs=4) as at_pool,
            tc.tile_pool(name="bigpool", bufs=1) as big_pool,
            tc.tile_pool(name="hpool", bufs=8) as h_pool,
            tc.tile_pool(name="stats", bufs=2) as st_pool,
            tc.tile_pool(name="small", bufs=3) as sm_pool,
        ):
            # ---------- weights / constants ----------
            ident = wp.tile([128, 128], F32)
            make_identity(nc, ident[:])
            wq4_sb = wp.tile([128, 2, 128], F32R)
            nc.sync.dma_start(
                wq4_sb[:], wq4_d.rearrange("(eb p) m -> p eb m", p=128).bitcast(F32R)
            )
            wk4_sb = wp.tile([128, 2, 128], F32R)
            nc.sync.dma_start(
                wk4_sb[:], wk4_d.rearrange("(eb p) m -> p eb m", p=128).bitcast(F32R)
            )
            wv_sb = wp.tile([128, 2, DK], F32R)
            nc.sync.dma_start(
                wv_sb[:], wv_d.rearrange("(eb p) m -> p eb m", p=128).bitcast(F32R)
            )
            wpf_sb = wp.tile([DK + 2, EMB + 2], F32R)
            nc.sync.dma_start(wpf_sb[:], wpf_d[:].bitcast(F32R))
            w1_sb = wp.tile([128, 2, HID], F32R)
            nc.sync.dma_start(
                w1_sb[:], w1_d.rearrange("(eb p) m -> p eb m", p=128).bitcast(F32R)
            )
            b1_sb = wp.tile([128, 8], F32)
            nc.sync.dma_start(b1_sb[:], b1_d[:])
            w2_sb = wp.tile([128, 8, EMB], F32R)
            nc.sync.dma_start(
                w2_sb[:], w2_d.rearrange("(hb p) m -> p hb m", p=128).bitcast(F32R)
            )
            if need_bcast or apply_b2:
                ones1_sb = wp.tile([1, 128], F32R)
                nc.vector.memset(ones1_sb[:].bitcast(I32), 0x3F800000)
                aff_sb = wp.tile([1, 5, EMB], F32R)
                nc.sync.dma_start(aff_sb[:], aff_d[:].bitcast(F32R))
            if need_bcast:
                with tc.tile_pool(name="psbc", bufs=1, space="PSUM") as psbc:
                    ps_b = psbc.tile([128, 4, EMB], F32, tag="bc")
                    for i in range(4):
                        nc.tensor.matmul(
                            ps_b[:, i], ones1_sb[:], aff_sb[:, 1 + i],
                            start=True, stop=True,
                        )
                    affb_sb = wp.tile([128, 4, EMB], F32)
                    nc.vector.tensor_copy(affb_sb[:], ps_b[:])

            def _emit_batches():
                for b in range(B_LOC):
                    # ---------- loads ----------
                    xt_sb = xq.tile([128, 2, T], F32R, tag="xt", bufs=2)
                    nc.sync.dma_start(
                        xt_sb[:],
                        xt_d[b].rearrange("(eb p) t -> p eb t", p=128).bitcast(F32R),
                    )
                    xp_sb = xq.tile([128, NT, EMB], F32, tag="xp", bufs=1)
                    nc.sync.dma_start(
                        xp_sb[:], xp_d[b].rearrange("(nt p) e -> p nt e", p=128)
                    )

                    if PH < 2:
                        nc.sync.dma_start(
                            out_d[b].rearrange("(nt p) e -> p nt e", p=128), xp_sb[:]
                        )
                        continue
                    qT_sb = qk_pool.tile([128, T], F32R, tag="qT", bufs=1)
                    kT_sb = qk_pool.tile([128, T], F32R, tag="kT", bufs=1)
                    v_ext = qk_pool.tile([128, NT, DK + 2], F32R, tag="v", bufs=1)
                    attn_sb = qk_pool.tile([DK + 2, T], F32R, tag="attn", bufs=1)

                    with tc.tile_pool(name="psatt", bufs=1, space="PSUM") as psatt:
                        # q,k projections (x4 replicated rows), per 512-chunk
                        for c in range(NCH):
                            for w4, dst in ((wq4_sb, qT_sb), (wk4_sb, kT_sb)):
                                ps_qk = psatt.tile(
                                    [128, 512], F32, tag="sc", bufs=4, name="ps_qk"
                                )
                                for eb in range(2):
                                    nc.tensor.matmul(
                                        ps_qk[:],
                                        w4[:, eb],
                                        xt_sb[:, eb, c * 512:(c + 1) * 512],
                                        start=(eb == 0), stop=(eb == 1),
                                    )
                                nc.scalar.copy(
                                    dst[:, c * 512:(c + 1) * 512], ps_qk[:]
                                )

                        # v projection (natural [s, dk]) + ones column
                        ps_v = psatt.tile([128, NT, DK], F32, tag="v", bufs=1)
                        for tb in range(NT):
                            for eb in range(2):
                                nc.tensor.matmul(
                                    ps_v[:, tb],
                                    xt_sb[:, eb, tb * 128:(tb + 1) * 128],
                                    wv_sb[:, eb],
                                    start=(eb == 0), stop=(eb == 1),
                                )
                        nc.vector.tensor_copy(v_ext[:, :, 0:DK], ps_v[:])
                        nc.vector.memset(v_ext[:, :, DK:DK + 1].bitcast(I32), 0x3F800000)
                        nc.vector.memset(v_ext[:, :, DK + 1:DK + 2].bitcast(I32), 0)

                        # attention: scoresT -> exp -> (diag mask) -> attn accum
                        for j in range(NCH):
                            t0 = j * 512
                            ps_at = psatt.tile([DK + 2, 512], F32, tag="at", bufs=2)
                            n_sb = 4 * j + 4
                            use_rt = not os.environ.get("KDBG_NO_RT")
                            for sb in range(n_sb):
                                lo = max(0, sb * 128 - t0)
                                grp = (sb % 4) * DK if use_rt else 0
                                ps_sc = psatt.tile([128, 512], F32, tag="sc", bufs=4)
                                nc.tensor.matmul(
                                    ps_sc[:, lo:512],
                                    kT_sb[grp:grp + DK, sb * 128:(sb + 1) * 128],
                                    qT_sb[grp:grp + DK, t0 + lo:t0 + 512],
                                    start=True, stop=True,
                                    tile_position=(grp, 0) if use_rt else None,
                                )
                                a_t = at_pool.tile([128, 512], F32R, tag="aT", bufs=4)
                                nc.scalar.activation(
                                    a_t[:, lo:512], ps_sc[:, lo:512], AF.Exp
                                )
                                if sb * 128 >= t0:  # diagonal block: causal mask
                                    nc.gpsimd.affine_select(
                                        out=a_t[:, lo:lo + 128],
                                        in_=a_t[:, lo:lo + 128],
                                        compare_op=OP.is_ge,
                                        fill=0.0,
                                        base=0,
                                        pattern=[[1, 128]],
                                        channel_multiplier=-1,
                                    )
                                nc.tensor.matmul(
                                    ps_at[:, lo:512],
                                    v_ext[:, sb, :],
                                    a_t[:, lo:512],
                                    start=(sb == 0), stop=(sb == n_sb - 1),
                                )
                            nc.vector.tensor_copy(attn_sb[:, t0:t0 + 512], ps_at[:])

                    if PH < 3:
                        nc.sync.dma_start(
                            out_d[b].rearrange("(nt p) e -> p nt e", p=128), xp_sb[:]
                        )
                        continue
                    # ---------- mh + z1 + LN1 + transpose ----------
                    z1_sb = big_pool.tile([128, NT, EMB], F32, tag="zres", bufs=1)
                    y1_sb = big_pool.tile([128, NT, EMB], F32, tag="y1", bufs=1)
                    recip = st_pool.tile([128, NT], F32, tag="recip", bufs=2)
                    y1T = [
                        big_pool.tile([128, T], F32R, tag=f"y1T{eb}", bufs=1, name=f"y1T{eb}")
                        for eb in range(2)
                    ]
                    with tc.tile_pool(name="psmh", bufs=1, space="PSUM") as psmh:
                        for tb in range(NT):
                            ps_mh = psmh.tile([128, EMB + 2], F32, tag="mh", bufs=4)
                            nc.tensor.matmul(
                                ps_mh[:],
                                attn_sb[:, tb * 128:(tb + 1) * 128],
                                wpf_sb[:],
                                start=True, stop=True,
                            )
                            nc.vector.reciprocal(
                                recip[:, tb:tb + 1], ps_mh[:, EMB:EMB + 1]
                            )
                            nc.vector.scalar_tensor_tensor(
                                out=z1_sb[:, tb],
                                in0=ps_mh[:, 0:EMB],
                                scalar=recip[:, tb:tb + 1],
                                in1=xp_sb[:, tb],
                                op0=OP.mult,
                                op1=OP.add,
                            )

                        _layernorm(nc, st_pool, sm_pool, z1_sb, y1_sb)
                        if apply_g1be1:
                            nc.vector.tensor_tensor(
                                y1_sb[:], y1_sb[:],
                                affb_sb[:, 0:1, :].to_broadcast([128, NT, EMB]),
                                OP.mult,
                            )
                            nc.vector.tensor_tensor(
                                y1_sb[:], y1_sb[:],
                                affb_sb[:, 1:2, :].to_broadcast([128, NT, EMB]),
                                OP.add,
                            )

                        for eb in range(2):
                            for half in range(2):
                                ps_tr = psmh.tile([128, 1024], F32, tag="tr", bufs=2)
                                for q in range(8):
                                    tb = half * 8 + q
                                    nc.tensor.transpose(
                                        ps_tr[:, q * 128:(q + 1) * 128],
                                        y1_sb[:, tb, eb * 128:(eb + 1) * 128],
                                        ident[:],
                                    )
                                nc.vector.tensor_copy(
                                    y1T[eb][:, half * 1024:(half + 1) * 1024], ps_tr[:]
                                )

                    if PH < 4:
                        nc.sync.dma_start(
                            out_d[b].rearrange("(nt p) e -> p nt e", p=128), y1_sb[:]
                        )
                        continue
                    # ---------- FFN + LN2 ----------
                    z2_sb = big_pool.tile([128, NT, EMB], F32, tag="zres", bufs=1)
                    y2_sb = big_pool.tile([128, NT, EMB], F32, tag="y2", bufs=1)
                    with tc.tile_pool(name="psffn", bufs=1, space="PSUM") as psffn:
                        for qtr in range(4):
                            hTg = [
                                h_pool.tile([128, 512], F32R, tag="hTg", bufs=8, name=f"hTg{_h}")
                                for _h in range(8)
                            ]
                            for h in range(8):
                                ps_h = psffn.tile([128, 512], F32, tag="h", bufs=2)
                                for eb in range(2):
                                    nc.tensor.matmul(
                                        ps_h[:],
                                        w1_sb[:, eb, h * 128:(h + 1) * 128],
                                        y1T[eb][:, qtr * 512:(qtr + 1) * 512],
                                        start=(eb == 0), stop=(eb == 1),
                                    )
                                nc.scalar.activation(
                                    hTg[h][:], ps_h[:], AF.Gelu, bias=b1_sb[:, h:h + 1]
                                )
                            ps_ff = psffn.tile([128, 4, EMB], F32, tag="ff", bufs=2)
                            for i in range(4):
                                if apply_b2:
                                    nc.tensor.matmul(
                                        ps_ff[:, i], ones1_sb[:], aff_sb[:, 0],
                                        start=True, stop=False,
                                    )
                                for h in range(8):
                                    tloc = i * 128
                                    nc.tensor.matmul(
                                        ps_ff[:, i],
                                        hTg[h][:, tloc:tloc + 128],
                                        w2_sb[:, h],
                                        start=(h == 0 and not apply_b2),
                                        stop=(h == 7),
                                    )
                            for i in range(4):
                                tb = qtr * 4 + i
                                nc.vector.scalar_tensor_tensor(
                                    out=z2_sb[:, tb],
                                    in0=ps_ff[:, i],
                                    scalar=1.0,
                                    in1=y1_sb[:, tb],
                                    op0=OP.mult,
                                    op1=OP.add,
                                )

                        _layernorm(nc, st_pool, sm_pool, z2_sb, y2_sb)
                        if apply_g2be2:
                            nc.vector.tensor_tensor(
                                y2_sb[:], y2_sb[:],
                                affb_sb[:, 2:3, :].to_broadcast([128, NT, EMB]),
                                OP.mult,
                            )
                            nc.vector.tensor_tensor(
                                y2_sb[:], y2_sb[:],
                                affb_sb[:, 3:4, :].to_broadcast([128, NT, EMB]),
                                OP.add,
                            )
                        nc.sync.dma_start(
                            out_d[b].rearrange("(nt p) e -> p nt e", p=128), y2_sb[:]
                        )

            LOOP_N = int(os.environ.get("KDBG_LOOP", "0"))
            if LOOP_N:
                with tc.For_i(0, LOOP_N, 1):
                    _emit_batches()
            else:
                _emit_batches()

    nc.compile()
    return nc


_CACHE = {}


def _get_nc(flags):
    if flags not in _CACHE:
        _CACHE[flags] = build_decoder(*flags)
    return _CACHE[flags]


def make_in_maps(x, Wq, Wk, Wv, Wp, bp, W1, b1, W2, b2, g1, be1, g2, be2):
    """Host-side preprocessing; returns per-core input maps + build flags."""
    f = np.asarray
    x = f(x, np.float32)
    wq4 = np.tile(f(Wq, np.float32) / math.sqrt(DK), (1, 4)).astype(np.float32)
    wk4 = np.tile(f(Wk, np.float32), (1, 4)).astype(np.float32)
    wpf = np.zeros((DK + 2, EMB + 2), np.float32)
    wpf[0:DK, 0:EMB] = f(Wp, np.float32).reshape(EMB // DK, DK, EMB).sum(axis=0)
    wpf[DK, EMB] = 1.0
    xp = (x + f(bp, np.float32)[None, None, :]).astype(np.float32)
    xt = np.ascontiguousarray(np.transpose(x, (0, 2, 1)))
    b1m = np.ascontiguousarray(f(b1, np.float32).reshape(8, 128).T)
    aff = np.stack(
        [f(b2), f(g1), f(be1), f(g2), f(be2)]
    ).astype(np.float32)[None]

    flags = (
        not (np.all(f(g1) == 1.0) and np.all(f(be1) == 0.0)),
        not (np.all(f(g2) == 1.0) and np.all(f(be2) == 0.0)),
        bool(np.any(f(b2) != 0.0)),
    )
    shared = {
        "wq4": wq4,
        "wk4": wk4,
        "wv": f(Wv, np.float32),
        "wpf": wpf,
        "w1": f(W1, np.float32),
        "b1": b1m,
        "w2": f(W2, np.float32),
        "aff": aff,
    }
    in_maps = []
    for c in range(N_CORES):
        sl = slice(c * B_LOC, (c + 1) * B_LOC)
        in_maps.append({"xp": xp[sl], "xt": xt[sl], **shared})
    return in_maps, flags


def kernel(**inputs) -> np.ndarray:
    in_maps, flags = make_in_maps(**inputs)
    nc = _get_nc(flags)
    res = run_bass_kernel_spmd(nc, in_maps, core_ids=list(range(N_CORES)))
    return np.concatenate([r["out"] for r in res.results], axis=0)



# revision 1
# speedup vs baseline: 7.4365x; 7.4365x over previous
"""Trainium2 Bass kernel for nn_Decoder_58377195487266.

Single-layer decoder: shared-head causal attention (d_k=32) + FFN(256->1024->256)
with two LayerNorms. B=16, T=2048, EMB=256.

Sharding: pure data-parallel over batch. 8 cores x 2 batches each, weights
replicated, no collectives.

Host-side algebraic folds (exact):
  - cat = tile(attn, 8)  =>  cat @ Wp == attn @ (sum of Wp's 8 row blocks).
  - 1/sqrt(d_k) score scale folded into Wq.
  - bp folded into the residual input (x + bp).
  - softmax denominator: ones-column appended to v + unit column appended to
    the folded Wp, so the mh matmul also emits sum_s exp(score) per token;
    normalization multiplies by its reciprocal during z1 assembly.
"""

import math
import os

import numpy as np

import concourse.bass as bass  # noqa: F401
import concourse.mybir as mybir
from concourse import bacc
from concourse.bass_utils import run_bass_kernel_spmd
from concourse.masks import make_identity
from concourse.tile import TileContext

F32 = mybir.dt.float32
F32R = mybir.dt.float32r
I32 = mybir.dt.int32
AF = mybir.ActivationFunctionType
OP = mybir.AluOpType

B, T, EMB = 16, 2048, 256
DK = 32
HID = 4 * EMB
N_CORES = 8
B_LOC = B // N_CORES  # 2
NT = T // 128         # 16 t-blocks
NCH = T // 512        # 4 chunks
LN_EPS = 1e-5

RSQRT_MAGIC = 0x5F3759DF


def _nr_rsqrt(nc, pool, out, varp):
    """out = 1/sqrt(varp) via bit-trick + 3 Newton iterations, all on DVE."""
    sh = list(varp.shape)
    yi = pool.tile(sh, I32, tag="nr_i", bufs=2)
    magic = pool.tile(sh, I32, tag="nr_m", bufs=2)
    nc.vector.memset(magic[:], RSQRT_MAGIC)
    nc.vector.tensor_scalar(yi[:], varp.bitcast(I32), 1, None, OP.logical_shift_right)
    nc.vector.tensor_tensor(yi[:], magic[:], yi[:], OP.subtract)
    y = yi.bitcast(F32)
    e = pool.tile(sh, F32, tag="nr_e", bufs=2)
    h = pool.tile(sh, F32, tag="nr_h", bufs=2)
    for _ in range(3):
        nc.vector.tensor_tensor(e[:], y[:], y[:], OP.mult)
        nc.vector.tensor_tensor(e[:], e[:], varp[:], OP.mult)
        nc.vector.tensor_scalar(h[:], e[:], -0.5, 1.5, OP.mult, OP.add)
        nc.vector.tensor_tensor(y[:], y[:], h[:], OP.mult)
    nc.vector.tensor_copy(out[:], y[:])


def _layernorm(nc, st_pool, sm_pool, z_sb, y_sb):
    """Natural-layout LN: stats from accumulated sums, NR rsqrt, gpsimd apply.

    z_sb: [128, NT, EMB] fp32. Writes normalized (no affine) y_sb.
    """
    stats6 = st_pool.tile([128, NT, 6], F32, tag="st6", bufs=2)
    agg = st_pool.tile([128, NT, 2], F32, tag="agg", bufs=2)
    for tb in range(NT):
        nc.vector.bn_stats(stats6[:, tb], z_sb[:, tb])
        nc.vector.bn_aggr(agg[:, tb], stats6[:, tb])
    mean = agg[:, :, 0]
    varp = st_pool.tile([128, NT], F32, tag="varp", bufs=2)
    rstd = st_pool.tile([128, NT], F32, tag="rstd", bufs=2)
    mrstd = st_pool.tile([128, NT], F32, tag="mrstd", bufs=2)
    nc.vector.tensor_scalar(varp[:], agg[:, :, 1], 1.0, LN_EPS, OP.mult, OP.add)
    _nr_rsqrt(nc, st_pool, rstd, varp)
    nc.vector.tensor_tensor(mrstd[:], mean, rstd[:], OP.mult)
    for tb in range(NT):
        eng = nc.vector if os.environ.get("KDBG_NO_GPSIMD") else nc.gpsimd
        eng.tensor_scalar(
            y_sb[:, tb], z_sb[:, tb],
            rstd[:, tb:tb + 1], mrstd[:, tb:tb + 1],
            OP.mult, OP.subtract,
        )


def build_decoder(apply_g1be1: bool, apply_g2be2: bool, apply_b2: bool):
    """Build the per-core Bass program (B_LOC batches, full T each)."""
    PH = int(os.environ.get("KDBG_PHASE", "99"))
    nc = bacc.Bacc(None, target_bir_lowering=False)

    xp_d = nc.dram_tensor("xp", [B_LOC, T, EMB], F32, kind="ExternalInput")
    xt_d = nc.dram_tensor("xt", [B_LOC, EMB, T], F32, kind="ExternalInput")
    wq4_d = nc.dram_tensor("wq4", [EMB, 128], F32, kind="ExternalInput")
    wk4_d = nc.dram_tensor("wk4", [EMB, 128], F32, kind="ExternalInput")
    wv_d = nc.dram_tensor("wv", [EMB, DK], F32, kind="ExternalInput")
    wpf_d = nc.dram_tensor("wpf", [DK + 2, EMB + 2], F32, kind="ExternalInput")
    w1_d = nc.dram_tensor("w1", [EMB, HID], F32, kind="ExternalInput")
    b1_d = nc.dram_tensor("b1", [128, 8], F32, kind="ExternalInput")
    w2_d = nc.dram_tensor("w2", [HID, EMB], F32, kind="ExternalInput")
    aff_d = nc.dram_tensor("aff", [1, 5, EMB], F32, kind="ExternalInput")
    # aff rows: b2, g1, be1, g2, be2
    out_d = nc.dram_tensor("out", [B_LOC, T, EMB], F32, kind="ExternalOutput")

    need_bcast = apply_g1be1 or apply_g2be2

    with TileContext(nc) as tc:
        with (
            tc.tile_pool(name="wpool", bufs=1) as wp,
            tc.tile_pool(name="xpool", bufs=2) as xq,
            tc.tile_pool(name="qkpool", bufs=1) as qk_pool,
            tc.tile_pool(name="atpool", bufs=4) as at_pool,
            tc.tile_pool(name="bigpool", bufs=1) as big_pool,
            tc.tile_pool(name="hpool", bufs=8) as h_pool,
            tc.tile_pool(name="stats", bufs=2) as st_pool,
            tc.tile_pool(name="small", bufs=3) as sm_pool,
        ):
            # ---------- weights / constants ----------
            ident = wp.tile([128, 128], F32)
            make_identity(nc, ident[:])
            wq4_sb = wp.tile([128, 2, 128], F32R)
            nc.sync.dma_start(
                wq4_sb[:], wq4_d.rearrange("(eb p) m -> p eb m", p=128).bitcast(F32R)
            )
            wk4_sb = wp.tile([128, 2, 128], F32R)
            nc.sync.dma_start(
                wk4_sb[:], wk4_d.rearrange("(eb p) m -> p eb m", p=128).bitcast(F32R)
            )
            wv_sb = wp.tile([128, 2, DK], F32R)
            nc.sync.dma_start(
                wv_sb[:], wv_d.rearrange("(eb p) m -> p eb m", p=128).bitcast(F32R)
            )
            wpf_sb = wp.tile([DK + 2, EMB + 2], F32R)
            nc.sync.dma_start(wpf_sb[:], wpf_d[:].bitcast(F32R))
            w1_sb = wp.tile([128, 2, HID], F32R)
            nc.sync.dma_start(
                w1_sb[:], w1_d.rearrange("(eb p) m -> p eb m", p=128).bitcast(F32R)
            )
            b1_sb = wp.tile([128, 8], F32)
            nc.sync.dma_start(b1_sb[:], b1_d[:])
            w2_sb = wp.tile([128, 8, EMB], F32R)
            nc.sync.dma_start(
                w2_sb[:], w2_d.rearrange("(hb p) m -> p hb m", p=128).bitcast(F32R)
            )
            if need_bcast or apply_b2:
                ones1_sb = wp.tile([1, 128], F32R)
                nc.vector.memset(ones1_sb[:].bitcast(I32), 0x3F800000)
                aff_sb = wp.tile([1, 5, EMB], F32R)
                nc.sync.dma_start(aff_sb[:], aff_d[:].bitcast(F32R))
            if need_bcast:
                with tc.tile_pool(name="psbc", bufs=1, space="PSUM") as psbc:
                    ps_b = psbc.tile([128, 4, EMB], F32, tag="bc")
                    for i in range(4):
                        nc.tensor.matmul(
                            ps_b[:, i], ones1_sb[:], aff_sb[:, 1 + i],
                            start=True, stop=True,
                        )
                    affb_sb = wp.tile([128, 4, EMB], F32)
                    nc.vector.tensor_copy(affb_sb[:], ps_b[:])

            def _emit_batches():
                for b in range(B_LOC):
                    # ---------- loads ----------
                    xt_sb = xq.tile([128, 2, T], F32R, tag="xt", bufs=2)
                    nc.sync.dma_start(
                        xt_sb[:],
                        xt_d[b].rearrange("(eb p) t -> p eb t", p=128).bitcast(F32R),
                    )
                    xp_sb = xq.tile([128, NT, EMB], F32, tag="xp", bufs=1)
                    nc.sync.dma_start(
                        xp_sb[:], xp_d[b].rearrange("(nt p) e -> p nt e", p=128)
                    )

                    if PH < 2:
                        nc.sync.dma_start(
                            out_d[b].rearrange("(nt p) e -> p nt e", p=128), xp_sb[:]
                        )
                        continue
                    qT_sb = qk_pool.tile([128, T], F32R, tag="qT", bufs=1)
                    kT_sb = qk_pool.tile([128, T], F32R, tag="kT", bufs=1)
                    v_ext = qk_pool.tile([128, NT, DK + 2], F32R, tag="v", bufs=1)
                    attn_sb = qk_pool.tile([DK + 2, T], F32R, tag="attn", bufs=1)

                    with tc.tile_pool(name="psatt", bufs=1, space="PSUM") as psatt:
                        # q,k projections (x4 replicated rows), per 512-chunk
                        for c in range(NCH):
                            for w4, dst in ((wq4_sb, qT_sb), (wk4_sb, kT_sb)):
                                ps_qk = psatt.tile(
                                    [128, 512], F32, tag="sc", bufs=4, name="ps_qk"
                                )
                                for eb in range(2):
                                    nc.tensor.matmul(
                                        ps_qk[:],
                                        w4[:, eb],
                                        xt_sb[:, eb, c * 512:(c + 1) * 512],
                                        start=(eb == 0), stop=(eb == 1),
                                    )
                                nc.scalar.copy(
                                    dst[:, c * 512:(c + 1) * 512], ps_qk[:]
                                )

                        # v projection (natural [s, dk]) + ones column
                        ps_v = psatt.tile([128, NT, DK], F32, tag="v", bufs=1)
                        for tb in range(NT):
                            for eb in range(2):
                                nc.tensor.matmul(
                                    ps_v[:, tb],
                                    xt_sb[:, eb, tb * 128:(tb + 1) * 128],
                                    wv_sb[:, eb],
                                    start=(eb == 0), stop=(eb == 1),
                                )
                        nc.vector.tensor_copy(v_ext[:, :, 0:DK], ps_v[:])
                        nc.vector.memset(v_ext[:, :, DK:DK + 1].bitcast(I32), 0x3F800000)
                        nc.vector.memset(v_ext[:, :, DK + 1:DK + 2].bitcast(I32), 0)

                        # attention: scoresT -> exp -> (diag mask) -> attn accum
                        for j in range(NCH):
                            t0 = j * 512
                            ps_at = psatt.tile([DK + 2, 512], F32, tag="at", bufs=2)
                            n_sb = 4 * j + 4
                            use_rt = not os.environ.get("KDBG_NO_RT")
                            for sb in range(n_sb):
                                lo = max(0, sb * 128 - t0)
                                grp = (sb % 4) * DK if use_rt else 0
                                ps_sc = psatt.tile([128, 512], F32, tag="sc", bufs=4)
                                nc.tensor.matmul(
                                    ps_sc[:, lo:512],
                                    kT_sb[grp:grp + DK, sb * 128:(sb + 1) * 128],
                                    qT_sb[grp:grp + DK, t0 + lo:t0 + 512],
                                    start=True, stop=True,
                                    tile_position=(grp, 0) if use_rt else None,
                                )
                                a_t = at_pool.tile([128, 512], F32R, tag="aT", bufs=4)
                                nc.scalar.activation(
                                    a_t[:, lo:512], ps_sc[:, lo:512], AF.Exp
                                )
                                if sb * 128 >= t0:  # diagonal block: causal mask
                                    nc.gpsimd.affine_select(
                                        out=a_t[:, lo:lo + 128],
                                        in_=a_t[:, lo:lo + 128],
                                        compare_op=OP.is_ge,
                                        fill=0.0,
                                        base=0,
                                        pattern=[[1, 128]],
                                        channel_multiplier=-1,
                                    )
                                nc.tensor.matmul(
                                    ps_at[:, lo:512],
                                    v_ext[:, sb, :],
                                    a_t[:, lo:512],
                                    start=(sb == 0), stop=(sb == n_sb - 1),
                                )
                            nc.vector.tensor_copy(attn_sb[:, t0:t0 + 512], ps_at[:])

                    if PH < 3:
                        nc.sync.dma_start(
                            out_d[b].rearrange("(nt p) e -> p nt e", p=128), xp_sb[:]
                        )
                        continue
                    # ---------- mh + z1 + LN1 + transpose ----------
                    z1_sb = big_pool.tile([128, NT, EMB], F32, tag="zres", bufs=1)
                    y1_sb = big_pool.tile([128, NT, EMB], F32, tag="y1", bufs=1)
                    recip = st_pool.tile([128, NT], F32, tag="recip", bufs=2)
                    y1T = [
                        big_pool.tile([128, T], F32R, tag=f"y1T{eb}", bufs=1, name=f"y1T{eb}")
                        for eb in range(2)
                    ]
                    with tc.tile_pool(name="psmh", bufs=1, space="PSUM") as psmh:
                        for tb in range(NT):
                            ps_mh = psmh.tile([128, EMB + 2], F32, tag="mh", bufs=4)
                            nc.tensor.matmul(
                                ps_mh[:],
                                attn_sb[:, tb * 128:(tb + 1) * 128],
                                wpf_sb[:],
                                start=True, stop=True,
                            )
                            nc.vector.reciprocal(
                                recip[:, tb:tb + 1], ps_mh[:, EMB:EMB + 1]
                            )
                            nc.vector.scalar_tensor_tensor(
                                out=z1_sb[:, tb],
                                in0=ps_mh[:, 0:EMB],
                                scalar=recip[:, tb:tb + 1],
                                in1=xp_sb[:, tb],
                                op0=OP.mult,
                                op1=OP.add,
                            )

                        _layernorm(nc, st_pool, sm_pool, z1_sb, y1_sb)
                        if apply_g1be1:
                            nc.vector.tensor_tensor(
                                y1_sb[:], y1_sb[:],
                                affb_sb[:, 0:1, :].to_broadcast([128, NT, EMB]),
                                OP.mult,
                            )
                            nc.vector.tensor_tensor(
                                y1_sb[:], y1_sb[:],
                                affb_sb[:, 1:2, :].to_broadcast([128, NT, EMB]),
                                OP.add,
                            )

                        for eb in range(2):
                            for half in range(2):
                                ps_tr = psmh.tile([128, 1024], F32, tag="tr", bufs=2)
                                for q in range(8):
                                    tb = half * 8 + q
                                    nc.tensor.transpose(
                                        ps_tr[:, q * 128:(q + 1) * 128],
                                        y1_sb[:, tb, eb * 128:(eb + 1) * 128],
                                        ident[:],
                                    )
                                nc.vector.tensor_copy(
                                    y1T[eb][:, half * 1024:(half + 1) * 1024], ps_tr[:]
                                )

                    if PH < 4:
                        nc.sync.dma_start(
                            out_d[b].rearrange("(nt p) e -> p nt e", p=128), y1_sb[:]
                        )
                        continue
                    # ---------- FFN + LN2 ----------
                    z2_sb = big_pool.tile([128, NT, EMB], F32, tag="zres", bufs=1)
                    y2_sb = big_pool.tile([128, NT, EMB], F32, tag="y2", bufs=1)
                    with tc.tile_pool(name="psffn", bufs=1, space="PSUM") as psffn:
                        for qtr in range(4):
                            hTg = [
                                h_pool.tile([128, 512], F32R, tag="hTg", bufs=8, name=f"hTg{_h}")
                                for _h in range(8)
                            ]
                            for h in range(8):
                                ps_h = psffn.tile([128, 512], F32, tag="h", bufs=2)
                                for eb in range(2):
                                    nc.tensor.matmul(
                                        ps_h[:],
                                        w1_sb[:, eb, h * 128:(h + 1) * 128],
                                        y1T[eb][:, qtr * 512:(qtr + 1) * 512],
                                        start=(eb == 0), stop=(eb == 1),
                                    )
                                nc.scalar.activation(
                                    hTg[h][:], ps_h[:], AF.Gelu, bias=b1_sb[:, h:h + 1]
                                )
                            ps_ff = psffn.tile([128, 4, EMB], F32, tag="ff", bufs=2)
                            for i in range(4):
                                if apply_b2:
                                    nc.tensor.matmul(
                                        ps_ff[:, i], ones1_sb[:], aff_sb[:, 0],
                                        start=True, stop=False,
                                    )
                                for h in range(8):
                                    tloc = i * 128
                                    nc.tensor.matmul(
                                        ps_ff[:, i],
                                        hTg[h][:, tloc:tloc + 128],
                                        w2_sb[:, h],
                                        start=(h == 0 and not apply_b2),
                                        stop=(h == 7),
                                    )
                            for i in range(4):
                                tb = qtr * 4 + i
                                nc.vector.scalar_tensor_tensor(
                                    out=z2_sb[:, tb],
                                    in0=ps_ff[:, i],
                                    scalar=1.0,
                                    in1=y1_sb[:, tb],
                                    op0=OP.mult,
                                    op1=OP.add,
                                )

                        _layernorm(nc, st_pool, sm_pool, z2_sb, y2_sb)
                        if apply_g2be2:
                            nc.vector.tensor_tensor(
                                y2_sb[:], y2_sb[:],
                                affb_sb[:, 2:3, :].to_broadcast([128, NT, EMB]),
                                OP.mult,
                            )
                            nc.vector.tensor_tensor(
                                y2_sb[:], y2_sb[:],
                                affb_sb[:, 3:4, :].to_broadcast([128, NT, EMB]),
                                OP.add,
                            )
                        nc.sync.dma_start(
                            out_d[b].rearrange("(nt p) e -> p nt e", p=128), y2_sb[:]
                        )

            LOOP_N = int(os.environ.get("KDBG_LOOP", "0"))
            if LOOP_N:
                with tc.For_i(0, LOOP_N, 1):
                    _emit_batches()
            else:
                _emit_batches()

    nc.compile()
    return nc


_CACHE = {}


def _get_nc(flags):
    if flags not in _CACHE:
        _CACHE[flags] = build_decoder(*flags)
    return _CACHE[flags]


def make_in_maps(x, Wq, Wk, Wv, Wp, bp, W1, b1, W2, b2, g1, be1, g2, be2):
    """Host-side preprocessing; returns per-core input maps + build flags."""
    f = np.asarray
    x = f(x, np.float32)
    wq4 = np.tile(f(Wq, np.float32) / math.sqrt(DK), (1, 4)).astype(np.float32)
    wk4 = np.tile(f(Wk, np.float32), (1, 4)).astype(np.float32)
    wpf = np.zeros((DK + 2, EMB + 2), np.float32)
    wpf[0:DK, 0:EMB] = f(Wp, np.float32).reshape(EMB // DK, DK, EMB).sum(axis=0)
    wpf[DK, EMB] = 1.0
    xp = (x + f(bp, np.float32)[None, None, :]).astype(np.float32)
    xt = np.ascontiguousarray(np.transpose(x, (0, 2, 1)))
    b1m = np.ascontiguousarray(f(b1, np.float32).reshape(8, 128).T)
    aff = np.stack(
        [f(b2), f(g1), f(be1), f(g2), f(be2)]
    ).astype(np.float32)[None]

    flags = (
        not (np.all(f(g1) == 1.0) and np.all(f(be1) == 0.0)),
        not (np.all(f(g2) == 1.0) and np.all(f(be2) == 0.0)),
        bool(np.any(f(b2) != 0.0)),
    )
    shared = {
        "wq4": wq4,
        "wk4": wk4,
        "wv": f(Wv, np.float32),
        "wpf": wpf,
        "w1": f(W1, np.float32),
        "b1": b1m,
        "w2": f(W2, np.float32),
        "aff": aff,
    }
    in_maps = []
    for c in range(N_CORES):
        sl = slice(c * B_LOC, (c + 1) * B_LOC)
        in_maps.append({"xp": xp[sl], "xt": xt[sl], **shared})
    return in_maps, flags


def kernel(**inputs) -> np.ndarray:
    in_maps, flags = make_in_maps(**inputs)
    nc = _get_nc(flags)
    res = run_bass_kernel_spmd(nc, in_maps, core_ids=list(range(N_CORES)))
    return np.concatenate([r["out"] for r in res.results], axis=0)

